# revision 14
# baseline (speedup 1.0000x reference)
"""Subject-routed batched matmul for Trainium2 (8 NeuronCores, SPMD data-parallel).

out[b, d, t] = sum_c x[b, c, t] * weights[subjects[b], c, d]

Strategy:
- Data-parallel over batch B=128 across 8 cores (16 batches each).
- Host-side: gather per-batch weights (weights[subjects], tiny), then split
  x and w into fp16 hi/lo pairs (x = hi + lo with lo = fp16(x - fp32(hi))).
  The pair represents fp32 to ~2^-24 relative, so the 3-term product
  hh + hl + lh on the PE is fp32-grade (measured rel err ~3e-7) while each
  matmul streams at 1 cycle/row (fp32 matmuls cost 4 cycles/row).
  hi+lo fp16 is 4 bytes/elem - same DMA bytes as fp32.
- Device: per batch, out[b] (256d, 2048t) = w[b].T @ x[b], tiled K=2x128
  (contraction over c), M=2x128 (d -> PSUM partitions), N=4x512 (t, one
  PSUM bank per tile). 6 matmuls per PSUM bank (3 products x 2 k-chunks).
- DMA: everything packed so each transfer is 2 MiB with >=8 KiB contiguous
  per partition. Loads on the SP HWDGE ring (nc.sync), stores on the ACT
  ring (nc.scalar) so they ride separate descriptor queues.
"""

import sys

for _p in ("/opt/trn_rl_repo", "/root/.axon_site/_ro/trn_rl_repo"):
    if _p not in sys.path:
        sys.path.append(_p)

import numpy as np

import concourse.mybir as mybir
import concourse.tile as tile
from concourse import bacc
from concourse.bass_utils import run_bass_kernel_spmd

B, C, D, T, N_SUBJECTS = 128, 256, 256, 2048, 8
N_CORES = 8
BPC = B // N_CORES  # batches per core

KC = C // 128  # k chunks (contraction dim on partitions)
MC = D // 128  # m chunks (output partition dim)
NT = 512       # n tile (one PSUM bank of f32)
NC_ = T // NT  # n chunks

F32 = mybir.dt.float32
F16 = mybir.dt.float16

# (w_half, x_half) products: hh + hl + lh  (lo*lo dropped, ~2^-24)
PRODUCTS = ((0, 0), (0, 1), (1, 0))

_compiled = None


def _build():
    nc = bacc.Bacc("TRN2", target_bir_lowering=False, debug=False)
    # x2[b, c, half, t] fp16 (half: 0=hi, 1=lo)
    # wp[p, b, k, half, d] fp16 — host-pre-packed to the SBUF layout so the
    # weight DMA is one fully contiguous 32 KiB/partition transfer (the
    # naive strided load needs 7680 512 B descriptors and a ~15 us HWDGE
    # dispatch that stalled the PE 22 us).
    x_d = nc.dram_tensor("x2", [BPC, C, 2, T], F16, kind="ExternalInput")
    w_d = nc.dram_tensor("wp", [128, BPC, KC, 2, D], F16, kind="ExternalInput")
    o_d = nc.dram_tensor("out", [BPC, D, T], F32, kind="ExternalOutput")

    with tile.TileContext(nc) as tc:
        with (
            tc.tile_pool(name="wpool", bufs=1) as wpool,
            tc.tile_pool(name="xpool", bufs=5) as xpool,
            tc.tile_pool(name="opool", bufs=3) as opool,
            tc.tile_pool(name="psum", bufs=8, space="PSUM") as psum,
        ):
            # Weights resident for the whole kernel (4 MiB, contiguous per
            # partition). b=0's slice loads separately so the first matmuls
            # start fast; both ride the GPSIMD SWDGE path, which competes
            # with neither the x loads (SP ring) nor the stores (ACT ring).
            wt0 = wpool.tile([128, 1, KC, 2, D], F16)
            wtr = wpool.tile([128, BPC - 1, KC, 2, D], F16)
            nc.gpsimd.dma_start(wt0[:], w_d[:, 0:1])
            nc.gpsimd.dma_start(wtr[:], w_d[:, 1:])

            for b in range(BPC):
                wt = wt0 if b == 0 else wtr
                wb = 0 if b == 0 else b - 1
                # xt[p, k, half, t] (2 MiB, one DMA)
                xt = xpool.tile([128, KC, 2, T], F16, tag="xt")
                nc.sync.dma_start(
                    xt[:], x_d[b].rearrange("(k p) h t -> p k h t", p=128)
                )
                for m in range(MC):
                    # ot[p, t] (1 MiB, stored as soon as this m is done)
                    ot = opool.tile([128, T], F32, tag="ot")
                    for n in range(NC_):
                        pt = psum.tile([128, NT], F32)
                        i = 0
                        last = len(PRODUCTS) * KC - 1
                        for (wh, xh) in PRODUCTS:
                            for k in range(KC):
                                nc.tensor.matmul(
                                    pt[:],
                                    wt[:, wb, k, wh, m * 128:(m + 1) * 128],
                                    xt[:, k, xh, n * NT:(n + 1) * NT],
                                    start=(i == 0),
                                    stop=(i == last),
                                )
                                i += 1
                        nc.vector.tensor_copy(ot[:, n * NT:(n + 1) * NT], pt[:])
                    # alternate stores across the ACT (HWDGE) and GPSIMD
                    # (SWDGE) rings; the GPSIMD ring is idle after the
                    # initial weight load
                    st_eng = nc.scalar if (b * MC + m) % 2 == 0 else nc.gpsimd
                    st_eng.dma_start(o_d[b, m * 128:(m + 1) * 128, :], ot[:])

    nc.compile()
    return nc


def _get_compiled():
    global _compiled
    if _compiled is None:
        _compiled = _build()
    return _compiled


def _split_f16(a):
    """a (fp32) -> interleaved (…, 2, last) fp16 hi/lo on a new axis -2."""
    hi = a.astype(np.float16)
    lo = (a - hi.astype(np.float32)).astype(np.float16)
    return np.stack([hi, lo], axis=-2)


def _run(x, subjects, weights, **spmd_kwargs):
    x = np.asarray(x, dtype=np.float32)
    subjects = np.asarray(subjects).astype(np.int64)
    weights = np.asarray(weights, dtype=np.float32)

    w_g = weights[subjects]                # (B, C, D) fp32
    x2 = _split_f16(x)                     # (B, C, 2, T) fp16
    w2 = _split_f16(w_g)                   # (B, C, 2, D) fp16
    # wp[core][p, b, k, half, d] = w2[core*BPC + b, k*128 + p, half, d]
    wp = np.ascontiguousarray(
        w2.reshape(N_CORES, BPC, KC, 128, 2, D).transpose(0, 3, 1, 2, 4, 5)
    )

    nc = _get_compiled()
    in_maps = [
        {
            "x2": x2[i * BPC:(i + 1) * BPC],
            "wp": wp[i],
        }
        for i in range(N_CORES)
    ]
    res = run_bass_kernel_spmd(
        nc, in_maps, core_ids=list(range(N_CORES)), **spmd_kwargs
    )
    out = np.concatenate([r["out"] for r in res.results], axis=0)
    return out, res


def kernel(x, subjects, weights):
    return _run(x, subjects, weights)[0]
